# revision 46
# baseline (speedup 1.0000x reference)
"""Trainium2 Bass kernel for a dense transformer block (B=8, T=1024, C=1024, H=16).

Data-parallel over batch across the 8 NeuronCores (one batch element per core,
weights replicated, no collectives).

v2 — restructured for PE density (HAM stays warm) and engine balance:
  - All h/h2 transposes via the XBAR DMA-transpose (off the PE).
  - Attention pipelined per head-pair c: the two heads' S matmuls (K=64) run
    concurrently in the PE array (row groups 0/64 via base partitions); the
    exp (Scalar) latency hides behind the interleaved QKT matmuls of pair c+1;
    causal diag masks run on the otherwise-idle GPSIMD; the softmax
    denominator (from an appended ones-column in the PV lhsT) is reciprocated
    per pair and broadcast via a DRAM round-trip DMA.
  - PSUM: QKT/PV share a 2-slot pool (4 banks), S uses 2 single-slot pools
    (4 banks) so the head pair lands in distinct banks.
  - fc2 keeps w_mlp SBUF-resident (loaded during fc1) instead of streaming it
    twice.
All matmuls bf16 (fp32 PSUM accumulation); LN statistics and the residual
stream stay fp32.
"""
import sys

sys.path.insert(0, "/opt/trn_rl_repo")

import numpy as np
import ml_dtypes

import concourse.bass as bass
import concourse.tile as tile
from concourse import mybir
from concourse.vector_clock import ScopedClock

F32 = mybir.dt.float32
BF16 = mybir.dt.bfloat16
AF = mybir.ActivationFunctionType

T, C, H, D = 1024, 1024, 16, 64
NT = T // 128   # 8 token chunks
NC_ = C // 128  # 8 feature chunks
EPS = 1e-5

# S^T group tiling: groups of key-strips j whose widths sum to <= 1024
# (one 2-bank PSUM tile, one exp per group). Strip j has width (8-j)*128.
S_GROUPS = [(0,), (1, 7), (2, 6), (3, 5), (4,)]

# ---------------------------------------------------------------------------
# Walrus in this container rejects >1 sem-wait per CTRL instruction; split the
# Tile tail-drain's waits across nop carriers.
_MAX_WAITS = 1


def _patched_drain_and_barrier(self, tick_clock, wait_clock):
    nc = self.nc
    carrier = nc.sync.nop(nofuse=True)
    wait_clock.add_sem_waits(carrier.ins, ScopedClock({None: tick_clock.global_clock}))
    si = carrier.ins.sync_info
    waits = list(si.on_wait) if si and si.on_wait else []
    if len(waits) > _MAX_WAITS:
        si.on_wait = waits[:_MAX_WAITS]
        for k in range(_MAX_WAITS, len(waits), _MAX_WAITS):
            extra = nc.sync.nop(nofuse=True)
            esi = extra.ins.sync_info
            if esi is None:
                extra.ins.sync_info = mybir.SyncInfo(
                    on_wait=waits[k:k + _MAX_WAITS], on_update=[]
                )
            else:
                esi.on_wait = waits[k:k + _MAX_WAITS]
    nc.sync.drain()
    nc.all_engine_barrier()
    popped = nc._tile_sem_poison_stack.pop()
    assert popped is self._sem_poison
    nc.clear_and_free_semaphores(list(self.sems.allocated().values()))
    nc.all_engine_barrier()


tile.TileContext._drain_and_barrier = _patched_drain_and_barrier


def _split_sync_waits(nc, max_waits=1):
    """Walrus here rejects >1 sem-wait per instruction; hoist extras onto
    preceding same-engine nops."""
    ctr = 0
    for f in nc.m.functions:
        for b in f.blocks:
            out = []
            for ins in b.instructions:
                si = ins.sync_info
                ws = list(si.on_wait) if si and si.on_wait else []
                if len(ws) > max_waits:
                    extra, keep = ws[:-max_waits], ws[-max_waits:]
                    for i in range(0, len(extra), max_waits):
                        nop = mybir.InstNoOp(
                            name=f"wsplit-{ctr}", engine=ins.engine,
                            sync_info=mybir.SyncInfo(
                                on_wait=extra[i:i + max_waits], on_update=[]))
                        ctr += 1
                        out.append(nop)
                    si.on_wait = keep
                out.append(ins)
            b.instructions = out


def build_nc(flags):
    nc = bass.Bass()

    x_d = nc.dram_tensor("x", [T, C], F32, kind="ExternalInput")
    # host-prearranged: [m_chunk, p, ko, 128] so per-chunk DMAs are contiguous
    wqk_d = nc.dram_tensor("w_qk", [2 * NC_, 128, NC_, 128], BF16,
                           kind="ExternalInput")
    wfc_d = nc.dram_tensor("w_fc", [4 * NC_, 128, NC_, 128], BF16,
                           kind="ExternalInput")
    wv_d = nc.dram_tensor("w_v", [C, C], BF16, kind="ExternalInput")
    wp_d = nc.dram_tensor("w_proj", [C, C], BF16, kind="ExternalInput")
    wmlp_d = nc.dram_tensor("w_mlp", [4 * C, C], BF16, kind="ExternalInput")
    mask_d = nc.dram_tensor("mask_ut", [128, 128], BF16, kind="ExternalInput")
    opt = {}
    if flags["b_qk"]:
        opt["b_qk"] = nc.dram_tensor("b_qk", [128, 2 * NC_], F32, kind="ExternalInput")
    if flags["b_v"]:
        opt["b_v"] = nc.dram_tensor("b_v", [C], F32, kind="ExternalInput")
    if flags["b_proj"]:
        opt["b_proj"] = nc.dram_tensor("b_proj", [C], F32, kind="ExternalInput")
    if flags["b_fc"]:
        opt["b_fc"] = nc.dram_tensor("b_fc", [128, 4 * NC_], F32, kind="ExternalInput")
    if flags["b_mlp"]:
        opt["b_mlp"] = nc.dram_tensor("b_mlp", [C], F32, kind="ExternalInput")
    for nm in ("ln1_g", "ln1_b", "ln2_g", "ln2_b"):
        if flags[nm]:
            opt[nm] = nc.dram_tensor(nm, [C], F32, kind="ExternalInput")
    out_d = nc.dram_tensor("out", [T, C], F32, kind="ExternalOutput")

    with tile.TileContext(nc) as tc:
        _build_body(nc, tc, flags, x_d, wqk_d, wv_d, wp_d, wfc_d, wmlp_d,
                    mask_d, opt, out_d)
    _split_sync_waits(nc)
    return nc


def _build_body(nc, tc, flags, x_d, wqk_d, wv_d, wp_d, wfc_d, wmlp_d,
                mask_d, opt, out_d):
    from contextlib import ExitStack

    ctx = ExitStack()
    with ctx:
        const = ctx.enter_context(tc.tile_pool(name="const", bufs=1))
        big = ctx.enter_context(tc.tile_pool(name="big", bufs=1))
        scratch = ctx.enter_context(tc.tile_pool(name="scratch", bufs=6))
        small = ctx.enter_context(tc.tile_pool(name="small", bufs=8))
        dram = ctx.enter_context(tc.tile_pool(name="dram", bufs=1, space="DRAM"))

        # ---- constants -----------------------------------------------------
        mask_sb = const.tile([128, 128], BF16, tag="mask")
        nc.sync.dma_start(mask_sb[:], mask_d[:])
        eps_t = const.tile([128, 1], F32, tag="eps")
        nc.vector.memset(eps_t[:], EPS)

        def rep128(vec_dram):
            t = const.tile([128, C], F32, tag=f"rep_{vec_dram.tensor.name}")
            src = bass.AP(tensor=vec_dram.tensor, offset=0, ap=[[0, 128], [1, C]])
            nc.gpsimd.dma_start(out=t[:], in_=src)
            return t

        ln1_g_rep = rep128(opt["ln1_g"]) if flags["ln1_g"] else None
        ln1_b_rep = rep128(opt["ln1_b"]) if flags["ln1_b"] else None
        ln2_g_rep = rep128(opt["ln2_g"]) if flags["ln2_g"] else None
        ln2_b_rep = rep128(opt["ln2_b"]) if flags["ln2_b"] else None
        bv_rep = rep128(opt["b_v"]) if flags["b_v"] else None
        bproj_rep = rep128(opt["b_proj"]) if flags["b_proj"] else None
        bmlp_rep = rep128(opt["b_mlp"]) if flags["b_mlp"] else None
        bqk_sb = None
        if flags["b_qk"]:
            bqk_sb = const.tile([128, 2 * NC_], F32, tag="bqk")
            nc.sync.dma_start(bqk_sb[:], opt["b_qk"][:])
        bfc_sb = None
        if flags["b_fc"]:
            bfc_sb = const.tile([128, 4 * NC_], F32, tag="bfc")
            nc.sync.dma_start(bfc_sb[:], opt["b_fc"][:])

        # ---- persistent tiles ---------------------------------------------
        x_sb = big.tile([128, NT, C], F32, tag="x")        # x, then r1 in place
        bufT = big.tile([128, NC_, T], BF16, tag="bufT")   # h1T -> h2T
        y_ctx = ExitStack()
        ypool = y_ctx.enter_context(tc.tile_pool(name="ypool", bufs=1))
        yT = ypool.tile([128, NC_, T], BF16, tag="yT")     # attention out^T
        wpp = ExitStack()
        wp_pool = wpp.enter_context(tc.tile_pool(name="wpp", bufs=1))
        wp_sb = wp_pool.tile([128, NC_, C], BF16, tag="wp")

        def layernorm_chunk(src_slice, g_rep, b_rep):
            stats = small.tile([128, 2, 6], F32, tag="bn_stats")
            xr = src_slice.rearrange("p (s f) -> p s f", f=512)
            for s in range(2):
                nc.vector.bn_stats(out=stats[:, s, :], in_=xr[:, s, :])
            mv = small.tile([128, 2], F32, tag="bn_mv")
            nc.vector.bn_aggr(out=mv[:], in_=stats[:])
            rstd = small.tile([128, 1], F32, tag="rstd")
            nc.scalar.activation(out=rstd[:], in_=mv[:, 1:2], func=AF.Sqrt,
                                 bias=eps_t[:], scale=1.0)
            nc.vector.reciprocal(out=rstd[:], in_=rstd[:])
            h_blk = scratch.tile([128, C], BF16, tag="h_blk")
            nc.vector.tensor_scalar(
                out=h_blk[:], in0=src_slice, scalar1=mv[:, 0:1], scalar2=rstd[:],
                op0=mybir.AluOpType.subtract, op1=mybir.AluOpType.mult)
            if g_rep is not None:
                nc.vector.tensor_mul(h_blk[:], h_blk[:], g_rep[:])
            if b_rep is not None:
                nc.vector.tensor_add(h_blk[:], h_blk[:], b_rep[:])
            return h_blk

        # ==== stage A+B: x load, LN1, DMA-transpose -> h1T, V ==============
        mid_ctx = ExitStack()
        mid = mid_ctx.enter_context(tc.tile_pool(name="mid", bufs=1))
        vaug = mid.tile([128, NT, H, D + 1], BF16, tag="vaug")
        nc.vector.memset(vaug[:, :, :, D:D + 1], 1.0)

        ab_ps = ExitStack()
        warm_pool = ab_ps.enter_context(
            tc.tile_pool(name="warm", bufs=1, space="PSUM"))
        vps = ab_ps.enter_context(tc.tile_pool(name="vps", bufs=2, space="PSUM"))

        warm = warm_pool.tile([128, 128], F32, tag="warm", name="warm")
        for _ in range(96):
            nc.tensor.matmul(warm[:], mask_sb[:], mask_sb[:], start=True,
                             stop=True)

        wvp = ExitStack()
        wv_pool = wvp.enter_context(tc.tile_pool(name="wvp", bufs=1))
        wv_sb = wv_pool.tile([128, NC_, C], BF16, tag="wv")
        # wv on the gpsimd (SWDGE) queue so the sync queue is a pure,
        # wait-free x-block prefetch stream
        for k in range(NC_):
            nc.gpsimd.dma_start(out=wv_sb[:, k, :],
                                in_=wv_d[k * 128:(k + 1) * 128, :])
        for ti in range(NT):
            nc.sync.dma_start(out=x_sb[:, ti, :],
                              in_=x_d[ti * 128:(ti + 1) * 128, :])

        def v_evict(ti, ps):
            # vaug eviction on Scalar (idle in this stage); emitted one block
            # late so its PE-completion wait doesn't clog the queue
            if bv_rep is not None:
                vs = scratch.tile([128, C], F32, tag="v_scr")
                nc.vector.tensor_add(vs[:], ps[:], bv_rep[:])
                vsrc = vs
            else:
                vsrc = ps
            nc.scalar.copy(
                out=vaug[:, ti, :, 0:D],
                in_=vsrc[:].rearrange("p (h d) -> p h d", d=D))

        pending_v = None
        for ti in range(NT):
            h_blk = layernorm_chunk(x_sb[:, ti, :], ln1_g_rep, ln1_b_rep)
            nc.sync.dma_start_transpose(
                bufT[:, :, ti * 128:(ti + 1) * 128], h_blk[:])
            # V(ti) = h(ti) @ Wv  (natural), augmented with ones col
            ps = vps.tile([128, C], F32, tag="vps", name=f"psv{ti}")
            for k in range(NC_):
                lhsT = bufT[:, k, ti * 128:(ti + 1) * 128]
                for off, n in ((0, 512), (512, 512)):
                    nc.tensor.matmul(ps[:, off:off + n], lhsT,
                                     wv_sb[:, k, off:off + n],
                                     start=(k == 0), stop=(k == NC_ - 1))
            if pending_v is not None:
                v_evict(*pending_v)
            pending_v = (ti, ps)
        v_evict(*pending_v)
        wvp.close()
        ab_ps.close()

        # prefetch w_proj now (fresh region, wait-free) — transfers overlap attn
        for k in range(NC_):
            nc.sync.dma_start(out=wp_sb[:, k, :],
                              in_=wp_d[k * 128:(k + 1) * 128, :])

        # ==== stages C+D: QKT + attention pipeline =========================
        att_ps = ExitStack()
        mm_pool = att_ps.enter_context(
            tc.tile_pool(name="mmp", bufs=2, space="PSUM"))
        sA_pool = att_ps.enter_context(
            tc.tile_pool(name="sA", bufs=1, space="PSUM"))
        sB_pool = att_ps.enter_context(
            tc.tile_pool(name="sB", bufs=1, space="PSUM"))

        att_sb = ExitStack()
        qk_pool = att_sb.enter_context(tc.tile_pool(name="qkp", bufs=2))
        wq_pool = att_sb.enter_context(tc.tile_pool(name="wqs", bufs=4))
        e_pool = att_sb.enter_context(tc.tile_pool(name="ep", bufs=12))
        scr_pool = att_sb.enter_context(tc.tile_pool(name="scrp", bufs=3))
        dnb_pool = att_sb.enter_context(tc.tile_pool(name="dnbp", bufs=1))
        rp_pool = att_sb.enter_context(tc.tile_pool(name="rpp", bufs=1))
        rbf_pool = att_sb.enter_context(tc.tile_pool(name="rbfp", bufs=2))
        den_dram = dram.tile([H, T], BF16)
        rec_dram = dram.tile([H, T], F32)

        qk_tiles = {}

        def emit_qkt_half(c, sub):
            """qkT chunk for head-pair c: sub=0 -> q rows, sub=1 -> k rows."""
            if sub == 0:
                qk_tiles[c] = qk_pool.tile([128, 2, T], BF16, tag="qkc",
                                           name=f"qkc_{c}")
            qk_c = qk_tiles[c]
            m = c + sub * NC_
            wq = wq_pool.tile([128, NC_, 128], BF16, tag="wq")
            nc.sync.dma_start(out=wq[:], in_=wqk_d[m])
            ps = mm_pool.tile([128, T], F32, tag="mmp", name=f"psqk{m}")
            for k in range(NC_):
                for off, n in ((0, 512), (512, 512)):
                    nc.tensor.matmul(ps[:, off:off + n], wq[:, k, :],
                                     bufT[:, k, off:off + n],
                                     start=(k == 0), stop=(k == NC_ - 1))
            if bqk_sb is not None:
                nc.scalar.activation(out=qk_c[:, sub, :], in_=ps[:],
                                     func=AF.Identity, bias=bqk_sb[:, m:m + 1])
            else:
                nc.vector.tensor_copy(out=qk_c[:, sub, :], in_=ps[:])

        def emit_s_pair(c, gi):
            """S^T group gi for both heads of pair c, packed in the PE array
            (row groups 0 / 64), then exp (Scalar) + causal mask (GPSIMD)."""
            grp = S_GROUPS[gi]
            w_g = sum((8 - j) * 128 for j in grp)
            qk_c = qk_tiles[c]
            out = {}
            for hh, pool in ((0, sA_pool), (1, sB_pool)):
                koff = hh * 64
                ps = pool.tile([128, w_g], F32, tag=f"s{hh}",
                               name=f"sp_{c}_{hh}_{gi}")
                col = 0
                offs = []
                for j in grp:
                    rem = (8 - j) * 128
                    lhsT = qk_c[koff:koff + 64, 1, j * 128:(j + 1) * 128]
                    off = col
                    src_off = j * 128
                    while off < col + rem:
                        n = min(col + rem - off, 512 - (off % 512))
                        nc.tensor.matmul(
                            ps[:, off:off + n], lhsT,
                            qk_c[koff:koff + 64, 0, src_off:src_off + n],
                            start=True, stop=True)
                        off += n
                        src_off += n
                    offs.append(col)
                    col += rem
                e = e_pool.tile([128, w_g], BF16, tag="e",
                                name=f"e_{c}_{hh}_{gi}")
                nc.scalar.activation(out=e[:], in_=ps[:], func=AF.Exp,
                                     scale=0.125)
                # causal diag masks: one GPSIMD op over the group's diag
                # blocks (equal 128-wide segments at offsets `offs`)
                if len(offs) == 1:
                    e_ap = e[:, offs[0]:offs[0] + 128]
                    m_ap = mask_sb[:]
                else:
                    stride = offs[1] - offs[0]
                    e_ap = bass.AP(tensor=e.tensor, offset=e[:].offset,
                                   ap=[e[:].ap[0], [stride, len(offs)],
                                       [1, 128]])
                    m_ap = bass.AP(tensor=mask_sb.tensor,
                                   offset=mask_sb[:].offset,
                                   ap=[mask_sb[:].ap[0], [0, len(offs)],
                                       [1, 128]])
                nc.gpsimd.tensor_mul(e_ap, e_ap, m_ap)
                out[hh] = (e, offs)
            return out

        def emit_pv(c, hh, egrps, j_range, ps_prev=None):
            """PV accumulation for head 2c+hh over key strips in j_range.
            egrps: j -> (e_tile, col offset in tile)."""
            koff = hh * 64
            if ps_prev is None:
                ps = mm_pool.tile([65, T], F32, tag="mmp", name=f"pv_{c}_{hh}")
            else:
                ps = ps_prev
            for j in j_range:
                lhsT = vaug[:, j, 2 * c + hh, :]
                et, eo = egrps[j]
                if j <= 3:
                    nA = (4 - j) * 128
                    nc.tensor.matmul(ps[:, j * 128:512], lhsT, et[:, eo:eo + nA],
                                     start=(j == 0), stop=(j == 3))
                    nc.tensor.matmul(ps[:, 512:1024], lhsT,
                                     et[:, eo + nA:eo + nA + 512],
                                     start=(j == 0), stop=False)
                else:
                    nB = (8 - j) * 128
                    nc.tensor.matmul(ps[:, j * 128:1024], lhsT,
                                     et[:, eo:eo + nB],
                                     start=False, stop=(j == NT - 1))
            return ps

        def finish_pv(c, hh, ps):
            koff = hh * 64
            scr = scr_pool.tile([65, T], BF16, tag="scr", name=f"scr_{c}_{hh}")
            nc.vector.tensor_copy(out=scr[:], in_=ps[:])
            nc.sync.dma_start(out=den_dram[2 * c + hh:2 * c + hh + 1, :],
                              in_=scr[64:65, :])
            nc.sync.dma_start(out=yT[koff:koff + 64, c, :], in_=scr[0:64, :])

        # deferred pair normalization (delay 2): the reciprocal is batched
        # over two pairs ([4, T] — DVE reciprocal cost is column-bound, so
        # batching halves the per-pair cost), round-trips DRAM to broadcast
        # over partitions, then a Vector multiply — far enough behind the
        # producing pair that nothing ever waits in the FIFO
        rbf_tiles = {}

        def recip_batch(p0, q=None):
            q = q or nc.sync
            dnb = dnb_pool.tile([4, T], BF16, tag="dnb", name=f"dnb_{p0}")
            q.dma_start(out=dnb[:], in_=den_dram[2 * p0:2 * p0 + 4, :])
            rp = rp_pool.tile([4, T], F32, tag="rp", name=f"rp_{p0}")
            nc.vector.reciprocal(out=rp[:], in_=dnb[:])
            q.dma_start(out=rec_dram[2 * p0:2 * p0 + 4, :], in_=rp[:])

        def norm_bcast(p, q=None):
            q = q or nc.sync
            rbf = rbf_pool.tile([128, T], F32, tag="rbf", name=f"rbf_{p}")
            bsrc = bass.AP(tensor=rec_dram.tensor, offset=2 * p * T,
                           ap=[[T, 2], [0, 64], [1, T]])
            q.dma_start(out=rbf[:], in_=bsrc)
            rbf_tiles[p] = rbf

        def norm_mul(p):
            nc.vector.tensor_mul(yT[:, p, :], yT[:, p, :], rbf_tiles.pop(p)[:])

        # prelude: QKT for pair 0
        emit_qkt_half(0, 0)
        emit_qkt_half(0, 1)

        for c in range(NC_):
            egrps = {0: {}, 1: {}}

            def absorb(gi):
                res = emit_s_pair(c, gi)
                for hh in (0, 1):
                    e, offs = res[hh]
                    for j, o in zip(S_GROUPS[gi], offs):
                        egrps[hh][j] = (e, o)

            absorb(0)
            if c + 1 < NC_:
                emit_qkt_half(c + 1, 0)
            if c >= 2 and c % 2 == 0:
                recip_batch(c - 2)
            absorb(1)
            absorb(2)
            if c + 1 < NC_:
                emit_qkt_half(c + 1, 1)
            if c >= 2:
                norm_bcast(c - 2)
            absorb(3)
            ps_h0 = emit_pv(c, 0, egrps[0], range(0, 3))
            absorb(4)
            emit_pv(c, 0, egrps[0], range(3, 8), ps_prev=ps_h0)
            finish_pv(c, 0, ps_h0)
            ps_h1 = emit_pv(c, 1, egrps[1], range(0, 8))
            finish_pv(c, 1, ps_h1)
            if c >= 2:
                norm_mul(c - 2)

        recip_batch(NC_ - 2, q=nc.scalar)
        for p in (NC_ - 2, NC_ - 1):
            norm_bcast(p, q=nc.scalar)
        for p in (NC_ - 2, NC_ - 1):
            norm_mul(p)
        att_sb.close()
        att_ps.close()
        mid_ctx.close()

        # ==== stage E: proj + residual + LN2 + DMA-transpose -> h2T ========
        e_ps = ExitStack()
        pj_pool = e_ps.enter_context(
            tc.tile_pool(name="pjp", bufs=3, space="PSUM"))

        def proj_mm(ps, i, ks, start):
            for k in ks:
                lhsT = yT[:, k, i * 128:(i + 1) * 128]
                for off, n in ((0, 512), (512, 512)):
                    nc.tensor.matmul(ps[:, off:off + n], lhsT,
                                     wp_sb[:, k, off:off + n],
                                     start=(start and k == ks[0]),
                                     stop=(k == NC_ - 1))

        def dmat2(i, h_blk):
            # scalar queue (sync carries the hoisted stage-F weight stream);
            # deferred 2 blocks so it never head-blocks the sqrt of the next
            # LN2 chain in the Scalar FIFO
            nc.scalar.dma_start_transpose(
                bufT[:, :, i * 128:(i + 1) * 128], h_blk[:])

        # blocks 0-2: accumulate k=0..5 first so the PE has work while the
        # last attention pairs (k=6,7) are still normalizing.
        partials = {}
        for i in range(3):
            ps = pj_pool.tile([128, C], F32, tag="pjp", name=f"pspj{i}")
            proj_mm(ps, i, list(range(6)), start=True)
            partials[i] = ps
        hb_pend = []
        for i in range(NT):
            if i in partials:
                ps = partials[i]
                proj_mm(ps, i, [6, 7], start=False)
            else:
                ps = pj_pool.tile([128, C], F32, tag="pjp", name=f"pspj{i}")
                proj_mm(ps, i, list(range(NC_)), start=True)
            nc.vector.tensor_add(x_sb[:, i, :], ps[:], x_sb[:, i, :])
            if bproj_rep is not None:
                nc.vector.tensor_add(x_sb[:, i, :], x_sb[:, i, :],
                                     bproj_rep[:])
            h_blk = layernorm_chunk(x_sb[:, i, :], ln2_g_rep, ln2_b_rep)
            hb_pend.append((i, h_blk))
            if len(hb_pend) > 2:
                dmat2(*hb_pend.pop(0))
        for args in hb_pend:
            dmat2(*args)
        e_ps.close()
        wpp.close()
        y_ctx.close()

        # ==== stage F: fc1 + gelu -> aT; preload w_mlp =====================
        fg_sb = ExitStack()
        at_pool = fg_sb.enter_context(tc.tile_pool(name="atp", bufs=1))
        wm_pool = fg_sb.enter_context(tc.tile_pool(name="wmp", bufs=1))
        f_ps = ExitStack()
        f1_pool = f_ps.enter_context(
            tc.tile_pool(name="f1p", bufs=3, space="PSUM"))
        aT = at_pool.tile([128, 4 * NC_, T], BF16, tag="aT")
        wmlp_sb = wm_pool.tile([128, 4 * NC_, C], BF16, tag="wmlp")
        with tc.tile_pool(name="wfcs", bufs=4) as wfcs:
            for m in range(4 * NC_):
                wf = wfcs.tile([128, NC_, 128], BF16, tag="wf")
                nc.sync.dma_start(out=wf[:], in_=wfc_d[m])
                # stream w_mlp into residence on the idle gpsimd (SWDGE) queue
                nc.gpsimd.dma_start(
                    out=wmlp_sb[:, m, :], in_=wmlp_d[m * 128:(m + 1) * 128, :])
                ps = f1_pool.tile([128, T], F32, tag="f1p", name=f"psf1_{m}")
                for k in range(NC_):
                    for off, n in ((0, 512), (512, 512)):
                        nc.tensor.matmul(ps[:, off:off + n], wf[:, k, :],
                                         bufT[:, k, off:off + n],
                                         start=(k == 0), stop=(k == NC_ - 1))
                bias = bfc_sb[:, m:m + 1] if bfc_sb is not None else 0.0
                nc.scalar.activation(out=aT[:, m, :], in_=ps[:],
                                     func=AF.Gelu_apprx_tanh, bias=bias)
        f_ps.close()

        # ==== stage G: fc2 + residual -> out ===============================
        with tc.tile_pool(name="ps_fc2", bufs=4, space="PSUM") as ps_fc2:
            for half in range(2):
                iis = list(range(half * 4, half * 4 + 4))
                psums = {}
                for i in iis:
                    psums[i] = ps_fc2.tile([128, C], F32, tag="psf2",
                                           name=f"psf2_{i}")
                for k in range(4 * NC_):
                    for i in iis:
                        lhsT = aT[:, k, i * 128:(i + 1) * 128]
                        for off, n in ((0, 512), (512, 512)):
                            nc.tensor.matmul(
                                psums[i][:, off:off + n], lhsT,
                                wmlp_sb[:, k, off:off + n],
                                start=(k == 0), stop=(k == 4 * NC_ - 1))
                for i in iis:
                    nc.vector.tensor_add(x_sb[:, i, :], psums[i][:],
                                         x_sb[:, i, :])
                    if bmlp_rep is not None:
                        nc.vector.tensor_add(x_sb[:, i, :], x_sb[:, i, :],
                                             bmlp_rep[:])
                    nc.sync.dma_start(out=out_d[i * 128:(i + 1) * 128, :],
                                      in_=x_sb[:, i, :])
        fg_sb.close()


# ---------------------------------------------------------------------------
_CACHE = {}


def _prearrange_kxm(w, nm):
    """[C, nm*128] -> [nm, 128, C//128, 128] bf16 so chunk DMAs are contiguous.

    out[m, p, ko, mm] = w[ko*128 + p, m*128 + mm]
    """
    cin = w.shape[0]
    a = w.reshape(cin // 128, 128, nm, 128)        # [ko, p, m, mm]
    a = np.transpose(a, (2, 1, 0, 3))              # [m, p, ko, mm]
    return np.ascontiguousarray(a.astype(ml_dtypes.bfloat16))


def _build_in_maps(inputs):
    x = np.asarray(inputs["x"], dtype=np.float32)
    w_qkv = np.asarray(inputs["w_qkv"], dtype=np.float32)
    b_qkv = np.asarray(inputs["b_qkv"], dtype=np.float32)

    flags = {
        "b_qk": bool(np.any(b_qkv[:2 * C])),
        "b_v": bool(np.any(b_qkv[2 * C:])),
        "b_proj": bool(np.any(inputs["b_attn_proj"])),
        "b_fc": bool(np.any(inputs["b_fc"])),
        "b_mlp": bool(np.any(inputs["b_mlp_proj"])),
        "ln1_g": not bool(np.allclose(np.asarray(inputs["ln1_g"]), 1.0)),
        "ln1_b": bool(np.any(inputs["ln1_b"])),
        "ln2_g": not bool(np.allclose(np.asarray(inputs["ln2_g"]), 1.0)),
        "ln2_b": bool(np.any(inputs["ln2_b"])),
    }

    bf = ml_dtypes.bfloat16
    shared = {
        "w_qk": _prearrange_kxm(w_qkv[:, :2 * C], 2 * NC_),
        "w_fc": _prearrange_kxm(np.asarray(inputs["w_fc"], np.float32), 4 * NC_),
        "w_v": np.ascontiguousarray(w_qkv[:, 2 * C:]).astype(bf),
        "w_proj": np.asarray(inputs["w_attn_proj"], np.float32).astype(bf),
        "w_mlp": np.asarray(inputs["w_mlp_proj"], np.float32).astype(bf),
        "mask_ut": np.triu(np.ones((128, 128))).astype(bf),
    }
    if flags["b_qk"]:
        shared["b_qk"] = np.ascontiguousarray(b_qkv[:2 * C].reshape(2 * NC_, 128).T)
    if flags["b_v"]:
        shared["b_v"] = np.ascontiguousarray(b_qkv[2 * C:])
    if flags["b_proj"]:
        shared["b_proj"] = np.asarray(inputs["b_attn_proj"], np.float32)
    if flags["b_fc"]:
        shared["b_fc"] = np.ascontiguousarray(
            np.asarray(inputs["b_fc"], np.float32).reshape(4 * NC_, 128).T)
    if flags["b_mlp"]:
        shared["b_mlp"] = np.asarray(inputs["b_mlp_proj"], np.float32)
    for nm in ("ln1_g", "ln1_b", "ln2_g", "ln2_b"):
        if flags[nm]:
            shared[nm] = np.asarray(inputs[nm], np.float32)

    in_maps = [dict(shared, x=np.ascontiguousarray(x[c])) for c in range(x.shape[0])]
    return flags, in_maps


def kernel_run(inputs, trace=False, trace_kwargs=None):
    """Build (cached), run on 8 cores, return (full_output, BassKernelResults)."""
    from concourse.bass_utils import run_bass_kernel_spmd

    flags, in_maps = _build_in_maps(inputs)
    key = tuple(sorted(flags.items()))
    if key not in _CACHE:
        _CACHE[key] = build_nc(flags)
    nc = _CACHE[key]
    res = run_bass_kernel_spmd(nc, in_maps, core_ids=list(range(8)),
                               trace=trace, trace_kwargs=trace_kwargs or {})
    out = np.stack([res.results[c]["out"] for c in range(8)]).astype(np.float32)
    return out, res


def kernel(**inputs) -> np.ndarray:
    out, _ = kernel_run(inputs, trace=False)
    return out


# revision 54
# speedup vs baseline: 1.0879x; 1.0879x over previous
"""Trainium2 Bass kernel for a dense transformer block (B=8, T=1024, C=1024, H=16).

Data-parallel over batch across the 8 NeuronCores (one batch element per core,
weights replicated, no collectives).

v2 — restructured for PE density (HAM stays warm) and engine balance:
  - All h/h2 transposes via the XBAR DMA-transpose (off the PE).
  - Attention pipelined per head-pair c: the two heads' S matmuls (K=64) run
    concurrently in the PE array (row groups 0/64 via base partitions); the
    exp (Scalar) latency hides behind the interleaved QKT matmuls of pair c+1;
    causal diag masks run on the otherwise-idle GPSIMD; the softmax
    denominator (from an appended ones-column in the PV lhsT) is reciprocated
    per pair and broadcast via a DRAM round-trip DMA.
  - PSUM: QKT/PV share a 2-slot pool (4 banks), S uses 2 single-slot pools
    (4 banks) so the head pair lands in distinct banks.
  - fc2 keeps w_mlp SBUF-resident (loaded during fc1) instead of streaming it
    twice.
All matmuls bf16 (fp32 PSUM accumulation); LN statistics and the residual
stream stay fp32.
"""
import sys

sys.path.insert(0, "/opt/trn_rl_repo")

import numpy as np
import ml_dtypes

import concourse.bass as bass
import concourse.tile as tile
from concourse import mybir
from concourse.masks import make_identity
from concourse.vector_clock import ScopedClock

F32 = mybir.dt.float32
BF16 = mybir.dt.bfloat16
AF = mybir.ActivationFunctionType

T, C, H, D = 1024, 1024, 16, 64
NT = T // 128   # 8 token chunks
NC_ = C // 128  # 8 feature chunks
EPS = 1e-5

# S^T group tiling: groups of key-strips j whose widths sum to <= 1024
# (one 2-bank PSUM tile, one exp per group). Strip j has width (8-j)*128.
S_GROUPS = [(0,), (1, 7), (2, 6), (3, 5), (4,)]

# ---------------------------------------------------------------------------
# Walrus in this container rejects >1 sem-wait per CTRL instruction; split the
# Tile tail-drain's waits across nop carriers.
_MAX_WAITS = 1


def _patched_drain_and_barrier(self, tick_clock, wait_clock):
    nc = self.nc
    carrier = nc.sync.nop(nofuse=True)
    wait_clock.add_sem_waits(carrier.ins, ScopedClock({None: tick_clock.global_clock}))
    si = carrier.ins.sync_info
    waits = list(si.on_wait) if si and si.on_wait else []
    if len(waits) > _MAX_WAITS:
        si.on_wait = waits[:_MAX_WAITS]
        for k in range(_MAX_WAITS, len(waits), _MAX_WAITS):
            extra = nc.sync.nop(nofuse=True)
            esi = extra.ins.sync_info
            if esi is None:
                extra.ins.sync_info = mybir.SyncInfo(
                    on_wait=waits[k:k + _MAX_WAITS], on_update=[]
                )
            else:
                esi.on_wait = waits[k:k + _MAX_WAITS]
    nc.sync.drain()
    nc.all_engine_barrier()
    popped = nc._tile_sem_poison_stack.pop()
    assert popped is self._sem_poison
    nc.clear_and_free_semaphores(list(self.sems.allocated().values()))
    nc.all_engine_barrier()


tile.TileContext._drain_and_barrier = _patched_drain_and_barrier


def _split_sync_waits(nc, max_waits=1):
    """Walrus here rejects >1 sem-wait per instruction; hoist extras onto
    preceding same-engine nops."""
    ctr = 0
    for f in nc.m.functions:
        for b in f.blocks:
            out = []
            for ins in b.instructions:
                si = ins.sync_info
                ws = list(si.on_wait) if si and si.on_wait else []
                if len(ws) > max_waits:
                    extra, keep = ws[:-max_waits], ws[-max_waits:]
                    for i in range(0, len(extra), max_waits):
                        nop = mybir.InstNoOp(
                            name=f"wsplit-{ctr}", engine=ins.engine,
                            sync_info=mybir.SyncInfo(
                                on_wait=extra[i:i + max_waits], on_update=[]))
                        ctr += 1
                        out.append(nop)
                    si.on_wait = keep
                out.append(ins)
            b.instructions = out


def build_nc(flags):
    nc = bass.Bass()

    x_d = nc.dram_tensor("x", [T, C], F32, kind="ExternalInput")
    # host-prearranged: [m_chunk, p, ko, 128] so per-chunk DMAs are contiguous
    wqk_d = nc.dram_tensor("w_qk", [2 * NC_, 128, NC_, 128], BF16,
                           kind="ExternalInput")
    wfc_d = nc.dram_tensor("w_fc", [4 * NC_, 128, NC_, 128], BF16,
                           kind="ExternalInput")
    wv_d = nc.dram_tensor("w_v", [C, C], BF16, kind="ExternalInput")
    wp_d = nc.dram_tensor("w_proj", [C, C], BF16, kind="ExternalInput")
    wmlp_d = nc.dram_tensor("w_mlp", [4 * C, C], BF16, kind="ExternalInput")
    mask_d = nc.dram_tensor("mask_ut", [128, 128], BF16, kind="ExternalInput")
    opt = {}
    if flags["b_qk"]:
        opt["b_qk"] = nc.dram_tensor("b_qk", [128, 2 * NC_], F32, kind="ExternalInput")
    if flags["b_v"]:
        opt["b_v"] = nc.dram_tensor("b_v", [C], F32, kind="ExternalInput")
    if flags["b_proj"]:
        opt["b_proj"] = nc.dram_tensor("b_proj", [C], F32, kind="ExternalInput")
    if flags["b_fc"]:
        opt["b_fc"] = nc.dram_tensor("b_fc", [128, 4 * NC_], F32, kind="ExternalInput")
    if flags["b_mlp"]:
        opt["b_mlp"] = nc.dram_tensor("b_mlp", [C], F32, kind="ExternalInput")
    for nm in ("ln1_g", "ln1_b", "ln2_g", "ln2_b"):
        if flags[nm]:
            opt[nm] = nc.dram_tensor(nm, [C], F32, kind="ExternalInput")
    out_d = nc.dram_tensor("out", [T, C], F32, kind="ExternalOutput")

    with tile.TileContext(nc) as tc:
        _build_body(nc, tc, flags, x_d, wqk_d, wv_d, wp_d, wfc_d, wmlp_d,
                    mask_d, opt, out_d)
    _split_sync_waits(nc)
    return nc


def _build_body(nc, tc, flags, x_d, wqk_d, wv_d, wp_d, wfc_d, wmlp_d,
                mask_d, opt, out_d):
    from contextlib import ExitStack

    ctx = ExitStack()
    with ctx:
        const = ctx.enter_context(tc.tile_pool(name="const", bufs=1))
        big = ctx.enter_context(tc.tile_pool(name="big", bufs=1))
        scratch = ctx.enter_context(tc.tile_pool(name="scratch", bufs=6))
        small = ctx.enter_context(tc.tile_pool(name="small", bufs=8))
        dram = ctx.enter_context(tc.tile_pool(name="dram", bufs=1, space="DRAM"))

        # ---- constants -----------------------------------------------------
        mask_sb = const.tile([128, 128], BF16, tag="mask")
        nc.sync.dma_start(mask_sb[:], mask_d[:])
        eps_t = const.tile([128, 1], F32, tag="eps")
        nc.vector.memset(eps_t[:], EPS)
        ident = const.tile([128, 128], BF16, tag="ident")
        make_identity(nc, ident)

        def rep128(vec_dram):
            t = const.tile([128, C], F32, tag=f"rep_{vec_dram.tensor.name}")
            src = bass.AP(tensor=vec_dram.tensor, offset=0, ap=[[0, 128], [1, C]])
            nc.gpsimd.dma_start(out=t[:], in_=src)
            return t

        ln1_g_rep = rep128(opt["ln1_g"]) if flags["ln1_g"] else None
        ln1_b_rep = rep128(opt["ln1_b"]) if flags["ln1_b"] else None
        ln2_g_rep = rep128(opt["ln2_g"]) if flags["ln2_g"] else None
        ln2_b_rep = rep128(opt["ln2_b"]) if flags["ln2_b"] else None
        bv_rep = rep128(opt["b_v"]) if flags["b_v"] else None
        bproj_rep = rep128(opt["b_proj"]) if flags["b_proj"] else None
        bmlp_rep = rep128(opt["b_mlp"]) if flags["b_mlp"] else None
        bqk_sb = None
        if flags["b_qk"]:
            bqk_sb = const.tile([128, 2 * NC_], F32, tag="bqk")
            nc.sync.dma_start(bqk_sb[:], opt["b_qk"][:])
        bfc_sb = None
        if flags["b_fc"]:
            bfc_sb = const.tile([128, 4 * NC_], F32, tag="bfc")
            nc.sync.dma_start(bfc_sb[:], opt["b_fc"][:])

        # ---- persistent tiles ---------------------------------------------
        x_sb = big.tile([128, NT, C], F32, tag="x")        # x, then r1 in place
        bufT = big.tile([128, NC_, T], BF16, tag="bufT")   # h1T -> h2T
        y_ctx = ExitStack()
        ypool = y_ctx.enter_context(tc.tile_pool(name="ypool", bufs=1))
        yT = ypool.tile([128, NC_, T], BF16, tag="yT")     # attention out^T
        wpp = ExitStack()
        wp_pool = wpp.enter_context(tc.tile_pool(name="wpp", bufs=1))
        wp_sb = wp_pool.tile([128, NC_, C], BF16, tag="wp")

        def layernorm_chunk(src_slice, g_rep, b_rep):
            stats = small.tile([128, 2, 6], F32, tag="bn_stats")
            xr = src_slice.rearrange("p (s f) -> p s f", f=512)
            for s in range(2):
                nc.vector.bn_stats(out=stats[:, s, :], in_=xr[:, s, :])
            mv = small.tile([128, 2], F32, tag="bn_mv")
            nc.vector.bn_aggr(out=mv[:], in_=stats[:])
            rstd = small.tile([128, 1], F32, tag="rstd")
            nc.scalar.activation(out=rstd[:], in_=mv[:, 1:2], func=AF.Sqrt,
                                 bias=eps_t[:], scale=1.0)
            nc.vector.reciprocal(out=rstd[:], in_=rstd[:])
            h_blk = scratch.tile([128, C], BF16, tag="h_blk")
            nc.vector.tensor_scalar(
                out=h_blk[:], in0=src_slice, scalar1=mv[:, 0:1], scalar2=rstd[:],
                op0=mybir.AluOpType.subtract, op1=mybir.AluOpType.mult)
            if g_rep is not None:
                nc.vector.tensor_mul(h_blk[:], h_blk[:], g_rep[:])
            if b_rep is not None:
                nc.vector.tensor_add(h_blk[:], h_blk[:], b_rep[:])
            return h_blk

        def transpose_into(tp_pool, dst_ti, src_blk):
            # PE transposes; evictions alternate Vector/Scalar to split load
            for jc in range(NC_):
                pst = tp_pool.tile([128, 128], BF16, tag="tp")
                nc.tensor.transpose(pst[:], src_blk[:, jc * 128:(jc + 1) * 128],
                                    ident[:])
                dst = bufT[:, jc, dst_ti * 128:(dst_ti + 1) * 128]
                if jc % 2 == 0:
                    nc.vector.tensor_copy(out=dst, in_=pst[:])
                else:
                    nc.scalar.copy(out=dst, in_=pst[:])

        # ==== stage A+B: x load, LN1, DMA-transpose -> h1T, V ==============
        mid_ctx = ExitStack()
        mid = mid_ctx.enter_context(tc.tile_pool(name="mid", bufs=1))
        vaug = mid.tile([128, NT, H, D + 1], BF16, tag="vaug")
        nc.vector.memset(vaug[:, :, :, D:D + 1], 1.0)

        ab_ps = ExitStack()
        warm_pool = ab_ps.enter_context(
            tc.tile_pool(name="warm", bufs=1, space="PSUM"))
        vps = ab_ps.enter_context(tc.tile_pool(name="vps", bufs=2, space="PSUM"))
        tp_ab = ab_ps.enter_context(tc.tile_pool(name="tpab", bufs=2,
                                                 space="PSUM"))

        warm = warm_pool.tile([128, 128], F32, tag="warm", name="warm")
        for _ in range(96):
            nc.tensor.matmul(warm[:], mask_sb[:], mask_sb[:], start=True,
                             stop=True)

        wvp = ExitStack()
        wv_pool = wvp.enter_context(tc.tile_pool(name="wvp", bufs=1))
        wv_sb = wv_pool.tile([128, NC_, C], BF16, tag="wv")
        # wv on the gpsimd (SWDGE) queue so the sync queue is a pure,
        # wait-free x-block prefetch stream
        for k in range(NC_):
            nc.gpsimd.dma_start(out=wv_sb[:, k, :],
                                in_=wv_d[k * 128:(k + 1) * 128, :])
        for ti in range(NT):
            nc.sync.dma_start(out=x_sb[:, ti, :],
                              in_=x_d[ti * 128:(ti + 1) * 128, :])

        def v_evict(ti, ps):
            # vaug eviction on Scalar (idle in this stage); emitted one block
            # late so its PE-completion wait doesn't clog the queue
            if bv_rep is not None:
                vs = scratch.tile([128, C], F32, tag="v_scr")
                nc.vector.tensor_add(vs[:], ps[:], bv_rep[:])
                vsrc = vs
            else:
                vsrc = ps
            nc.scalar.copy(
                out=vaug[:, ti, :, 0:D],
                in_=vsrc[:].rearrange("p (h d) -> p h d", d=D))

        pending_v = None
        for ti in range(NT):
            h_blk = layernorm_chunk(x_sb[:, ti, :], ln1_g_rep, ln1_b_rep)
            transpose_into(tp_ab, ti, h_blk[:])
            # V(ti) = h(ti) @ Wv  (natural), augmented with ones col
            ps = vps.tile([128, C], F32, tag="vps", name=f"psv{ti}")
            for k in range(NC_):
                lhsT = bufT[:, k, ti * 128:(ti + 1) * 128]
                for off, n in ((0, 512), (512, 512)):
                    nc.tensor.matmul(ps[:, off:off + n], lhsT,
                                     wv_sb[:, k, off:off + n],
                                     start=(k == 0), stop=(k == NC_ - 1))
            if pending_v is not None:
                v_evict(*pending_v)
            pending_v = (ti, ps)
        v_evict(*pending_v)
        wvp.close()
        ab_ps.close()

        # prefetch w_proj now (fresh region, wait-free) — transfers overlap attn
        for k in range(NC_):
            nc.sync.dma_start(out=wp_sb[:, k, :],
                              in_=wp_d[k * 128:(k + 1) * 128, :])

        # ==== stages C+D: QKT + attention pipeline =========================
        att_ps = ExitStack()
        mm_pool = att_ps.enter_context(
            tc.tile_pool(name="mmp", bufs=2, space="PSUM"))
        sA_pool = att_ps.enter_context(
            tc.tile_pool(name="sA", bufs=1, space="PSUM"))
        sB_pool = att_ps.enter_context(
            tc.tile_pool(name="sB", bufs=1, space="PSUM"))

        att_sb = ExitStack()
        qk_pool = att_sb.enter_context(tc.tile_pool(name="qkp", bufs=2))
        wq_pool = att_sb.enter_context(tc.tile_pool(name="wqs", bufs=4))
        e_pool = att_sb.enter_context(tc.tile_pool(name="ep", bufs=12))
        scr_pool = att_sb.enter_context(tc.tile_pool(name="scrp", bufs=3))
        dnb_pool = att_sb.enter_context(tc.tile_pool(name="dnbp", bufs=1))
        rp_pool = att_sb.enter_context(tc.tile_pool(name="rpp", bufs=1))
        rbf_pool = att_sb.enter_context(tc.tile_pool(name="rbfp", bufs=2))
        den_dram = dram.tile([H, T], BF16)
        rec_dram = dram.tile([H, T], F32)

        qk_tiles = {}

        def emit_qkt_half(c, sub):
            """qkT chunk for head-pair c: sub=0 -> q rows, sub=1 -> k rows."""
            if sub == 0:
                qk_tiles[c] = qk_pool.tile([128, 2, T], BF16, tag="qkc",
                                           name=f"qkc_{c}")
            qk_c = qk_tiles[c]
            m = c + sub * NC_
            wq = wq_pool.tile([128, NC_, 128], BF16, tag="wq")
            nc.sync.dma_start(out=wq[:], in_=wqk_d[m])
            ps = mm_pool.tile([128, T], F32, tag="mmp", name=f"psqk{m}")
            for k in range(NC_):
                for off, n in ((0, 512), (512, 512)):
                    nc.tensor.matmul(ps[:, off:off + n], wq[:, k, :],
                                     bufT[:, k, off:off + n],
                                     start=(k == 0), stop=(k == NC_ - 1))
            if bqk_sb is not None:
                nc.scalar.activation(out=qk_c[:, sub, :], in_=ps[:],
                                     func=AF.Identity, bias=bqk_sb[:, m:m + 1])
            else:
                nc.vector.tensor_copy(out=qk_c[:, sub, :], in_=ps[:])

        def emit_s_pair(c, gi):
            """S^T group gi for both heads of pair c, packed in the PE array
            (row groups 0 / 64), then exp (Scalar) + causal mask (GPSIMD)."""
            grp = S_GROUPS[gi]
            w_g = sum((8 - j) * 128 for j in grp)
            qk_c = qk_tiles[c]
            out = {}
            for hh, pool in ((0, sA_pool), (1, sB_pool)):
                koff = hh * 64
                ps = pool.tile([128, w_g], F32, tag=f"s{hh}",
                               name=f"sp_{c}_{hh}_{gi}")
                col = 0
                offs = []
                for j in grp:
                    rem = (8 - j) * 128
                    lhsT = qk_c[koff:koff + 64, 1, j * 128:(j + 1) * 128]
                    off = col
                    src_off = j * 128
                    while off < col + rem:
                        n = min(col + rem - off, 512 - (off % 512))
                        nc.tensor.matmul(
                            ps[:, off:off + n], lhsT,
                            qk_c[koff:koff + 64, 0, src_off:src_off + n],
                            start=True, stop=True)
                        off += n
                        src_off += n
                    offs.append(col)
                    col += rem
                e = e_pool.tile([128, w_g], BF16, tag="e",
                                name=f"e_{c}_{hh}_{gi}")
                nc.scalar.activation(out=e[:], in_=ps[:], func=AF.Exp,
                                     scale=0.125)
                # causal diag masks: one GPSIMD op over the group's diag
                # blocks (equal 128-wide segments at offsets `offs`)
                if len(offs) == 1:
                    e_ap = e[:, offs[0]:offs[0] + 128]
                    m_ap = mask_sb[:]
                else:
                    stride = offs[1] - offs[0]
                    e_ap = bass.AP(tensor=e.tensor, offset=e[:].offset,
                                   ap=[e[:].ap[0], [stride, len(offs)],
                                       [1, 128]])
                    m_ap = bass.AP(tensor=mask_sb.tensor,
                                   offset=mask_sb[:].offset,
                                   ap=[mask_sb[:].ap[0], [0, len(offs)],
                                       [1, 128]])
                nc.gpsimd.tensor_mul(e_ap, e_ap, m_ap)
                out[hh] = (e, offs)
            return out

        def emit_pv(c, hh, egrps, j_range, ps_prev=None):
            """PV accumulation for head 2c+hh over key strips in j_range.
            egrps: j -> (e_tile, col offset in tile)."""
            koff = hh * 64
            if ps_prev is None:
                ps = mm_pool.tile([65, T], F32, tag="mmp", name=f"pv_{c}_{hh}")
            else:
                ps = ps_prev
            for j in j_range:
                lhsT = vaug[:, j, 2 * c + hh, :]
                et, eo = egrps[j]
                if j <= 3:
                    nA = (4 - j) * 128
                    nc.tensor.matmul(ps[:, j * 128:512], lhsT, et[:, eo:eo + nA],
                                     start=(j == 0), stop=(j == 3))
                    nc.tensor.matmul(ps[:, 512:1024], lhsT,
                                     et[:, eo + nA:eo + nA + 512],
                                     start=(j == 0), stop=False)
                else:
                    nB = (8 - j) * 128
                    nc.tensor.matmul(ps[:, j * 128:1024], lhsT,
                                     et[:, eo:eo + nB],
                                     start=False, stop=(j == NT - 1))
            return ps

        def finish_pv(c, hh, ps):
            koff = hh * 64
            scr = scr_pool.tile([65, T], BF16, tag="scr", name=f"scr_{c}_{hh}")
            nc.vector.tensor_copy(out=scr[:], in_=ps[:])
            nc.sync.dma_start(out=den_dram[2 * c + hh:2 * c + hh + 1, :],
                              in_=scr[64:65, :])
            nc.sync.dma_start(out=yT[koff:koff + 64, c, :], in_=scr[0:64, :])

        # deferred pair normalization (delay 2): the reciprocal is batched
        # over two pairs ([4, T] — DVE reciprocal cost is column-bound, so
        # batching halves the per-pair cost), round-trips DRAM to broadcast
        # over partitions, then a Vector multiply — far enough behind the
        # producing pair that nothing ever waits in the FIFO
        rbf_tiles = {}

        def recip_batch(p0, q=None):
            q = q or nc.sync
            dnb = dnb_pool.tile([4, T], BF16, tag="dnb", name=f"dnb_{p0}")
            q.dma_start(out=dnb[:], in_=den_dram[2 * p0:2 * p0 + 4, :])
            rp = rp_pool.tile([4, T], F32, tag="rp", name=f"rp_{p0}")
            nc.vector.reciprocal(out=rp[:], in_=dnb[:])
            q.dma_start(out=rec_dram[2 * p0:2 * p0 + 4, :], in_=rp[:])

        def norm_bcast(p, q=None):
            q = q or nc.sync
            rbf = rbf_pool.tile([128, T], F32, tag="rbf", name=f"rbf_{p}")
            bsrc = bass.AP(tensor=rec_dram.tensor, offset=2 * p * T,
                           ap=[[T, 2], [0, 64], [1, T]])
            q.dma_start(out=rbf[:], in_=bsrc)
            rbf_tiles[p] = rbf

        def norm_mul(p):
            nc.vector.tensor_mul(yT[:, p, :], yT[:, p, :], rbf_tiles.pop(p)[:])

        # prelude: QKT for pair 0
        emit_qkt_half(0, 0)
        emit_qkt_half(0, 1)

        for c in range(NC_):
            egrps = {0: {}, 1: {}}

            def absorb(gi):
                res = emit_s_pair(c, gi)
                for hh in (0, 1):
                    e, offs = res[hh]
                    for j, o in zip(S_GROUPS[gi], offs):
                        egrps[hh][j] = (e, o)

            absorb(0)
            if c + 1 < NC_:
                emit_qkt_half(c + 1, 0)
            if c >= 2 and c % 2 == 0:
                recip_batch(c - 2)
            absorb(1)
            absorb(2)
            if c + 1 < NC_:
                emit_qkt_half(c + 1, 1)
            if c >= 2:
                norm_bcast(c - 2)
            absorb(3)
            ps_h0 = emit_pv(c, 0, egrps[0], range(0, 3))
            absorb(4)
            emit_pv(c, 0, egrps[0], range(3, 8), ps_prev=ps_h0)
            finish_pv(c, 0, ps_h0)
            ps_h1 = emit_pv(c, 1, egrps[1], range(0, 8))
            finish_pv(c, 1, ps_h1)
            if c >= 2:
                norm_mul(c - 2)

        recip_batch(NC_ - 2, q=nc.scalar)
        for p in (NC_ - 2, NC_ - 1):
            norm_bcast(p, q=nc.scalar)
        for p in (NC_ - 2, NC_ - 1):
            norm_mul(p)
        att_sb.close()
        att_ps.close()
        mid_ctx.close()

        # ==== stage E: proj + residual + LN2 + PE transpose -> h2T =========
        e_ps = ExitStack()
        pj_pool = e_ps.enter_context(
            tc.tile_pool(name="pjp", bufs=3, space="PSUM"))
        tp_e = e_ps.enter_context(tc.tile_pool(name="tpe", bufs=2,
                                               space="PSUM"))

        def proj_mm(ps, i, ks, start):
            for k in ks:
                lhsT = yT[:, k, i * 128:(i + 1) * 128]
                for off, n in ((0, 512), (512, 512)):
                    nc.tensor.matmul(ps[:, off:off + n], lhsT,
                                     wp_sb[:, k, off:off + n],
                                     start=(start and k == ks[0]),
                                     stop=(k == NC_ - 1))

        def dmat2(i, h_blk):
            transpose_into(tp_e, i, h_blk[:])

        # blocks 0-2: accumulate k=0..5 first so the PE has work while the
        # last attention pairs (k=6,7) are still normalizing.
        partials = {}
        for i in range(3):
            ps = pj_pool.tile([128, C], F32, tag="pjp", name=f"pspj{i}")
            proj_mm(ps, i, list(range(6)), start=True)
            partials[i] = ps
        hb_pend = []
        for i in range(NT):
            if i in partials:
                ps = partials[i]
                proj_mm(ps, i, [6, 7], start=False)
            else:
                ps = pj_pool.tile([128, C], F32, tag="pjp", name=f"pspj{i}")
                proj_mm(ps, i, list(range(NC_)), start=True)
            nc.vector.tensor_add(x_sb[:, i, :], ps[:], x_sb[:, i, :])
            if bproj_rep is not None:
                nc.vector.tensor_add(x_sb[:, i, :], x_sb[:, i, :],
                                     bproj_rep[:])
            h_blk = layernorm_chunk(x_sb[:, i, :], ln2_g_rep, ln2_b_rep)
            hb_pend.append((i, h_blk))
            if len(hb_pend) > 2:
                dmat2(*hb_pend.pop(0))
        for args in hb_pend:
            dmat2(*args)
        e_ps.close()
        wpp.close()
        y_ctx.close()

        # ==== stage F: fc1 + gelu -> aT; preload w_mlp =====================
        fg_sb = ExitStack()
        at_pool = fg_sb.enter_context(tc.tile_pool(name="atp", bufs=1))
        wm_pool = fg_sb.enter_context(tc.tile_pool(name="wmp", bufs=1))
        f_ps = ExitStack()
        f1_pool = f_ps.enter_context(
            tc.tile_pool(name="f1p", bufs=3, space="PSUM"))
        aT = at_pool.tile([128, 4 * NC_, T], BF16, tag="aT")
        wmlp_sb = wm_pool.tile([128, 4 * NC_, C], BF16, tag="wmlp")
        with tc.tile_pool(name="wfcs", bufs=4) as wfcs:
            for m in range(4 * NC_):
                wf = wfcs.tile([128, NC_, 128], BF16, tag="wf")
                nc.sync.dma_start(out=wf[:], in_=wfc_d[m])
                # stream w_mlp into residence on the idle gpsimd (SWDGE) queue
                nc.gpsimd.dma_start(
                    out=wmlp_sb[:, m, :], in_=wmlp_d[m * 128:(m + 1) * 128, :])
                ps = f1_pool.tile([128, T], F32, tag="f1p", name=f"psf1_{m}")
                for k in range(NC_):
                    for off, n in ((0, 512), (512, 512)):
                        nc.tensor.matmul(ps[:, off:off + n], wf[:, k, :],
                                         bufT[:, k, off:off + n],
                                         start=(k == 0), stop=(k == NC_ - 1))
                bias = bfc_sb[:, m:m + 1] if bfc_sb is not None else 0.0
                nc.scalar.activation(out=aT[:, m, :], in_=ps[:],
                                     func=AF.Gelu_apprx_tanh, bias=bias)
        f_ps.close()

        # ==== stage G: fc2 + residual -> out ===============================
        with tc.tile_pool(name="ps_fc2", bufs=4, space="PSUM") as ps_fc2:
            for half in range(2):
                iis = list(range(half * 4, half * 4 + 4))
                psums = {}
                for i in iis:
                    psums[i] = ps_fc2.tile([128, C], F32, tag="psf2",
                                           name=f"psf2_{i}")
                for k in range(4 * NC_):
                    for i in iis:
                        lhsT = aT[:, k, i * 128:(i + 1) * 128]
                        for off, n in ((0, 512), (512, 512)):
                            nc.tensor.matmul(
                                psums[i][:, off:off + n], lhsT,
                                wmlp_sb[:, k, off:off + n],
                                start=(k == 0), stop=(k == 4 * NC_ - 1))
                for i in iis:
                    nc.vector.tensor_add(x_sb[:, i, :], psums[i][:],
                                         x_sb[:, i, :])
                    if bmlp_rep is not None:
                        nc.vector.tensor_add(x_sb[:, i, :], x_sb[:, i, :],
                                             bmlp_rep[:])
                    nc.sync.dma_start(out=out_d[i * 128:(i + 1) * 128, :],
                                      in_=x_sb[:, i, :])
        fg_sb.close()


# ---------------------------------------------------------------------------
_CACHE = {}


def _prearrange_kxm(w, nm):
    """[C, nm*128] -> [nm, 128, C//128, 128] bf16 so chunk DMAs are contiguous.

    out[m, p, ko, mm] = w[ko*128 + p, m*128 + mm]
    """
    cin = w.shape[0]
    a = w.reshape(cin // 128, 128, nm, 128)        # [ko, p, m, mm]
    a = np.transpose(a, (2, 1, 0, 3))              # [m, p, ko, mm]
    return np.ascontiguousarray(a.astype(ml_dtypes.bfloat16))


def _build_in_maps(inputs):
    x = np.asarray(inputs["x"], dtype=np.float32)
    w_qkv = np.asarray(inputs["w_qkv"], dtype=np.float32)
    b_qkv = np.asarray(inputs["b_qkv"], dtype=np.float32)

    flags = {
        "b_qk": bool(np.any(b_qkv[:2 * C])),
        "b_v": bool(np.any(b_qkv[2 * C:])),
        "b_proj": bool(np.any(inputs["b_attn_proj"])),
        "b_fc": bool(np.any(inputs["b_fc"])),
        "b_mlp": bool(np.any(inputs["b_mlp_proj"])),
        "ln1_g": not bool(np.allclose(np.asarray(inputs["ln1_g"]), 1.0)),
        "ln1_b": bool(np.any(inputs["ln1_b"])),
        "ln2_g": not bool(np.allclose(np.asarray(inputs["ln2_g"]), 1.0)),
        "ln2_b": bool(np.any(inputs["ln2_b"])),
    }

    bf = ml_dtypes.bfloat16
    shared = {
        "w_qk": _prearrange_kxm(w_qkv[:, :2 * C], 2 * NC_),
        "w_fc": _prearrange_kxm(np.asarray(inputs["w_fc"], np.float32), 4 * NC_),
        "w_v": np.ascontiguousarray(w_qkv[:, 2 * C:]).astype(bf),
        "w_proj": np.asarray(inputs["w_attn_proj"], np.float32).astype(bf),
        "w_mlp": np.asarray(inputs["w_mlp_proj"], np.float32).astype(bf),
        "mask_ut": np.triu(np.ones((128, 128))).astype(bf),
    }
    if flags["b_qk"]:
        shared["b_qk"] = np.ascontiguousarray(b_qkv[:2 * C].reshape(2 * NC_, 128).T)
    if flags["b_v"]:
        shared["b_v"] = np.ascontiguousarray(b_qkv[2 * C:])
    if flags["b_proj"]:
        shared["b_proj"] = np.asarray(inputs["b_attn_proj"], np.float32)
    if flags["b_fc"]:
        shared["b_fc"] = np.ascontiguousarray(
            np.asarray(inputs["b_fc"], np.float32).reshape(4 * NC_, 128).T)
    if flags["b_mlp"]:
        shared["b_mlp"] = np.asarray(inputs["b_mlp_proj"], np.float32)
    for nm in ("ln1_g", "ln1_b", "ln2_g", "ln2_b"):
        if flags[nm]:
            shared[nm] = np.asarray(inputs[nm], np.float32)

    in_maps = [dict(shared, x=np.ascontiguousarray(x[c])) for c in range(x.shape[0])]
    return flags, in_maps


def kernel_run(inputs, trace=False, trace_kwargs=None):
    """Build (cached), run on 8 cores, return (full_output, BassKernelResults)."""
    from concourse.bass_utils import run_bass_kernel_spmd

    flags, in_maps = _build_in_maps(inputs)
    key = tuple(sorted(flags.items()))
    if key not in _CACHE:
        _CACHE[key] = build_nc(flags)
    nc = _CACHE[key]
    res = run_bass_kernel_spmd(nc, in_maps, core_ids=list(range(8)),
                               trace=trace, trace_kwargs=trace_kwargs or {})
    out = np.stack([res.results[c]["out"] for c in range(8)]).astype(np.float32)
    return out, res


def kernel(**inputs) -> np.ndarray:
    out, _ = kernel_run(inputs, trace=False)
    return out
